# revision 19
# baseline (speedup 1.0000x reference)
"""Trainium2 Bass kernel for a DoReFa-quantized ResNet BasicBlock.

    out = act(bn2(conv3x3(act(bn1(conv3x3(x, qw(w1)))), qw(w2))) + x)

with 4-bit DoReFa weight/activation quantization and training-mode BatchNorm
(batch statistics over N,H,W).

Strategy (8 NeuronCores, data-parallel over batch):
 - batch N=64 sharded 8 images/core; weights replicated.
 - BN uses *synced* batch statistics: per-core per-channel mean/var come from
   one-pass vector bn_stats/bn_aggr, converted to batch-mean contributions and
   AllGathered across the 8 cores (two tiny [128,2] collectives).
 - conv3x3 = 9 shifted matmuls accumulated in PSUM (C_in on partitions,
   pixels on free dim), two 8-row chunks per 2-bank PSUM tile so each
   PSUM evacuation moves 896 pixels in one ACT copy.
 - Weight quantization produces small odd integers (2m-15, |.| <= 15) which
   are exact in bf16/f16/fp8; the /15 scales are folded into the BN affine
   maps. conv1 runs f16 (x rounded once to f16), conv2 runs fp8 exactly with
   DoubleRow perf mode fusing tap pairs.
 - All intermediate images (conv outputs T1/T2, padded x) are kept in f16:
   2-byte DVE ops run at 2-4x, the padded f16 x doubles as the phase-3
   residual source (no HBM re-read), and the f16 rounding error is far below
   the quantization tolerance.
 - Activation quantization uses a +1024 f16 magic-constant round-to-nearest-
   even (f16 ulp is exactly 1 on [1024,2048)): u = min(y,15)+1024 -> f16,
   then ACT applies Relu(u-1024) while converting to fp8 codes (negatives,
   which round in coarser f16 ulp regions, land <= 0 and are clamped anyway).
 - Final output: out = Relu(u/15 - 1024/15) in one ACT pass (error ~1e-5,
   far below the 1/15 quantization step).
"""

import numpy as np

import bass_rust
import concourse.bacc as bacc
import concourse.mybir as mybir
import concourse.tile as tile
import concourse.bass_isa as bass_isa
from concourse.bass_utils import run_bass_kernel_spmd
from concourse.bass_interp import get_hw_module
from concourse.masks import make_identity

F32 = mybir.dt.float32
BF16 = mybir.dt.bfloat16
F16 = mybir.dt.float16
FP8 = mybir.dt.float8e4
AF = mybir.ActivationFunctionType
ALU = mybir.AluOpType
DR = mybir.MatmulPerfMode.DoubleRow

N_CORES = 8
N_PER = 8            # images per core
C = 128              # channels
H = W = 56
HW = H * W           # 3136
HHW = HW // 2        # half image pixels (28 rows)
PW = 58              # padded height/width
RCH = 8              # output rows per chunk
NCHUNK = H // RCH    # 7 chunks per image
CHN = RCH * W        # 448 pixels per chunk
NREC = N_PER * NCHUNK
MAGIC32 = float(2.0 ** 23)
MAGIC16 = 1024.0                      # f16 ulp == 1 on [1024, 2048)
N_BATCH = 64 * HW                     # full-batch BN sample count
# the padded image holds 15*x (so it doubles as the phase-3 residual), so
# conv1's output is on a 225*conv scale: eps pre-scaled by 225^2
EPS1 = float(np.float32(50625e-5))    # (15*15)^2 * 1e-5 (conv1 output scale)
EPS2 = float(np.float32(50625e-5))    # 225^2 * 1e-5  (conv2 output scale)
INV15 = float(np.float32(1.0 / 15.0))
P3BIAS = float(-(np.float32(1024.0) * np.float32(1.0 / 15.0)))
# tanh(w) ~ w * (1 + w2*(c1 + w2*(c2 + w2*c3))), |w| < 0.25
TC1 = float(np.float32(-1.0 / 3.0))
TC2 = float(np.float32(2.0 / 15.0))
TC3 = float(np.float32(-17.0 / 315.0))
TAPS = [(ky, kx) for ky in range(3) for kx in range(3)]
TAP_OFF = [ky * PW + kx for ky, kx in TAPS]

_CACHED = {}


def _pair_rhs(apad_ap, r0, p):
    """Overlapping AP selecting the two shifted conv windows of tap pair p:
    [C, 2, RCH, W] where dim1 steps between tap offsets (DoubleRow rhs)."""
    ky0, kx0 = TAPS[2 * p]
    delta = TAP_OFF[2 * p + 1] - TAP_OFF[2 * p]
    base = apad_ap[:, r0 + ky0:r0 + ky0 + RCH, kx0:kx0 + W]
    u = base.unsqueeze(1).broadcast_to((C, 2, RCH, W)).copy()
    pairs = [tuple(x) for x in u.ap]
    pairs[1] = (delta, 2)
    u.ap = bass_rust.VecI64Pair(pairs)
    return u


def _border_zero(nc, pad_ap):
    """Zero just the 1-px border ring of a [C, PW, PW] padded tile (the
    interior is fully overwritten)."""
    nc.vector.memset(pad_ap[:, 0:1, :], 0.0)
    nc.vector.memset(pad_ap[:, PW - 1:PW, :], 0.0)
    nc.vector.memset(pad_ap[:, 1:PW - 1, 0:1], 0.0)
    nc.vector.memset(pad_ap[:, 1:PW - 1, PW - 1:PW], 0.0)


def _tanh_poly_multi(nc, parts):
    """wt = taylor_tanh(w) elementwise over several (out, tmp, w2, w) slice
    groups, ops interleaved across groups to hide DVE inter-op latency."""
    steps = [
        lambda o, t, w2, w: nc.vector.tensor_tensor(w2, w, w, ALU.mult),
        lambda o, t, w2, w: nc.vector.tensor_scalar(t, w2, TC3, TC2,
                                                    ALU.mult, ALU.add),
        lambda o, t, w2, w: nc.vector.tensor_tensor(t, t, w2, ALU.mult),
        lambda o, t, w2, w: nc.vector.tensor_scalar(t, t, TC1, None, ALU.add),
        lambda o, t, w2, w: nc.vector.tensor_tensor(t, t, w2, ALU.mult),
        lambda o, t, w2, w: nc.vector.tensor_tensor(o, w, t, ALU.mult),
        lambda o, t, w2, w: nc.vector.tensor_tensor(o, w, o, ALU.add),
    ]
    for step in steps:
        for grp in parts:
            step(*grp)


def _tanh_poly(nc, tt_out, ts_tmp, w2src, wsrc):
    _tanh_poly_multi(nc, [(tt_out, ts_tmp, w2src, wsrc)])


def _quant_stats(nc, consts, wsb, name):
    """Global max |w| across all partitions (the only gpsimd step of the
    weight quant - hoisted early so it is not stuck behind the warm-up
    collective trigger, which blocks the gpsimd queue until the NEFF init
    barrier completes).  Reduced per column-half so each half overlaps its
    own DMA."""
    K = wsb.shape[-1]
    amax2 = consts.tile([C, 2], F32, tag=f"amax2{name}")
    nc.vector.tensor_reduce(amax2[:, 0:1], wsb[:, 0:K // 2],
                            mybir.AxisListType.X, ALU.max,
                            apply_absolute_value=True)
    nc.vector.tensor_reduce(amax2[:, 1:2], wsb[:, K // 2:],
                            mybir.AxisListType.X, ALU.max,
                            apply_absolute_value=True)
    amax = consts.tile([C, 1], F32, tag=f"amax{name}")
    nc.vector.tensor_reduce(amax[:], amax2[:], mybir.AxisListType.X, ALU.max)
    gmax = consts.tile([C, 1], F32, tag=f"gmax{name}")
    nc.gpsimd.partition_all_reduce(gmax[:], amax[:], C, bass_isa.ReduceOp.max)
    return gmax


def _quant_chain(nc, wqp, consts, wsb, gmax, name):
    """DoReFa-quantize one [128,128,3,3] weight (already DMA'd into wsb
    [C, C*9]): pure DVE chain producing the bf16 integer-code tile wi.
    Two interleaved half-column chains hide DVE inter-op latency; scratch
    A/B is reused along the chain to save SBUF."""
    K = C * 9
    # tanh(max) (tanh is monotone; same f32 poly as below)
    mt1 = consts.tile([C, 1], F32, tag=f"mt1{name}")
    mt2 = consts.tile([C, 1], F32, tag=f"mt2{name}")
    mval = consts.tile([C, 1], F32, tag=f"mval{name}")
    _tanh_poly(nc, mval[:], mt1[:], mt2[:], gmax[:])
    # s15 = 15 / (2*M); wn15 = wt*s15 + 7.5; codes = rtne(wn15)
    inv2m = consts.tile([C, 1], F32, tag=f"inv2m{name}")
    nc.vector.tensor_scalar(inv2m[:], mval[:], 2.0, None, ALU.mult)
    nc.vector.reciprocal(inv2m[:], inv2m[:])
    s15 = consts.tile([C, 1], F32, tag=f"s15{name}")
    nc.vector.tensor_scalar(s15[:], inv2m[:], 15.0, None, ALU.mult)
    A = wqp.tile([C, K], F32, tag="wqA")
    B = wqp.tile([C, K], F32, tag="wqB")
    wi = wqp.tile([C, K], BF16, tag=f"wi{name}")
    HC = K // 2
    halves = [slice(0, HC), slice(HC, K)]
    # tanh poly with STT fusion: wt = w*(1+p(w2)) in 5 ops
    steps = [
        lambda a, b, w: nc.vector.tensor_tensor(a, w, w, ALU.mult),
        lambda a, b, w: nc.vector.tensor_scalar(b, a, TC3, TC2,
                                                ALU.mult, ALU.add),
        lambda a, b, w: nc.vector.tensor_tensor(b, b, a, ALU.mult),
        lambda a, b, w: nc.vector.scalar_tensor_tensor(b, b, TC1, a,
                                                       ALU.add, ALU.mult),
        lambda a, b, w: nc.vector.scalar_tensor_tensor(a, b, 1.0, w,
                                                       ALU.add, ALU.mult),
        lambda a, b, w: nc.vector.tensor_scalar(a, a, s15[:, 0:1], 7.5,
                                                ALU.mult, ALU.add),
        lambda a, b, w: nc.vector.tensor_scalar(a, a, MAGIC32, -MAGIC32,
                                                ALU.add, ALU.add),
    ]
    for step in steps:
        for hs in halves:
            step(A[:, hs], B[:, hs], wsb[:, hs])
    for hs in halves:
        nc.vector.tensor_scalar(wi[:, hs], A[:, hs], 2.0, -15.0,
                                ALU.mult, ALU.add)
    return wi


def _quant_transposes(nc, ptr, ident, wi, copy_fn):
    """Transpose each tap via PE: lhsT[i, o] = Wi[o, i*9+t]."""
    wir = wi.rearrange("o (i t) -> o i t", t=9)
    for t in range(9):
        pst = ptr.tile([C, C], BF16, tag="tr")
        nc.tensor.transpose(pst[:], wir[:, :, t], ident[:])
        copy_fn(t, pst)


def _stats_contrib(nc, statsp, stats_rec, frac, name):
    """bn_aggr [C, k, 6] records into batch-mean contributions [C,2]:
    [mean, E[x^2]] scaled by frac = covered_samples / N_BATCH."""
    mv = statsp.tile([C, 2], F32, tag=f"mv{name}")
    nc.vector.bn_aggr(mv[:], stats_rec)
    msq = statsp.tile([C, 1], F32, tag=f"msq{name}")
    nc.vector.tensor_tensor(msq[:], mv[:, 0:1], mv[:, 0:1], ALU.mult)
    st = statsp.tile([C, 2], F32, tag=f"st{name}")
    nc.vector.tensor_scalar(st[:, 0:1], mv[:, 0:1], frac, None, ALU.mult)
    nc.vector.tensor_tensor(msq[:], mv[:, 1:2], msq[:], ALU.add)
    nc.vector.tensor_scalar(st[:, 1:2], msq[:], frac, None, ALU.mult)
    return st


def _ag_sum(nc, statsp, dram, st, RG, name):
    """Cross-core sum of a [C,2] stats tile via AllGather + local reduce.
    The input staging DMA rides the gpsimd queue - same in-order queue as
    the collective trigger, so no cross-queue semaphore hop."""
    agi = dram.tile([C, 2], F32, tag=f"agi{name}")
    ago = dram.tile([N_CORES, C, 2], F32, tag=f"ago{name}")
    nc.gpsimd.dma_start(agi[:], st[:])
    nc.gpsimd.collective_compute(
        "AllGather", ALU.bypass, replica_groups=RG,
        ins=[agi.opt()], outs=[ago.opt()])
    allst = statsp.tile([C, 2, N_CORES], F32, tag=f"allst{name}")
    nc.sync.dma_start(allst[:], ago.rearrange("r c s -> c s r"))
    rst = statsp.tile([C, 2], F32, tag=f"rst{name}")
    nc.vector.tensor_reduce(rst[:], allst[:], mybir.AxisListType.X, ALU.add)
    return rst


def _load_gb(nc, consts, gamma_ap, beta_ap, name):
    """Preload 15*gamma / 15*beta on the scalar DMA queue."""
    g = consts.tile([C, 1], F32, tag=f"g{name}")
    nc.scalar.dma_start(g[:], gamma_ap.rearrange("(c one) -> c one", one=1))
    b = consts.tile([C, 1], F32, tag=f"b{name}")
    nc.scalar.dma_start(b[:], beta_ap.rearrange("(c one) -> c one", one=1))
    g15 = consts.tile([C, 1], F32, tag=f"g15{name}")
    nc.vector.tensor_scalar(g15[:], g[:], 15.0, None, ALU.mult)
    b15 = consts.tile([C, 1], F32, tag=f"b15{name}")
    nc.vector.tensor_scalar(b15[:], b[:], 15.0, None, ALU.mult)
    return g15, b15


def _bn_vectors(nc, consts, rst, g15, b15, eps, name):
    """Build per-channel scale/bias [128,1] s.t. T*scale + bias equals
    15 * batchnorm(T/k); eps is pre-scaled by k^2.  rst = [mean, E[x^2]]
    on the T scale.  Minimal serial chain: 5 DVE ops + 1 ACT rsqrt."""
    msq = consts.tile([C, 1], F32, tag=f"msq{name}")
    nc.vector.tensor_tensor(msq[:], rst[:, 0:1], rst[:, 0:1], ALU.mult)
    var = consts.tile([C, 1], F32, tag=f"var{name}")
    nc.vector.tensor_tensor(var[:], rst[:, 1:2], msq[:], ALU.subtract)
    inv = consts.tile([C, 1], F32, tag=f"inv{name}")
    nc.scalar.activation(inv[:], var[:], AF.Sqrt, bias=eps[:, 0:1], scale=1.0)
    nc.vector.reciprocal(inv[:], inv[:])
    scale = consts.tile([C, 1], F32, tag=f"scale{name}")
    nc.vector.tensor_tensor(scale[:], g15[:], inv[:], ALU.mult)
    bias = consts.tile([C, 1], F32, tag=f"bias{name}")
    nc.vector.tensor_tensor(bias[:], rst[:, 0:1], scale[:], ALU.mult)
    nc.vector.tensor_tensor(bias[:], b15[:], bias[:], ALU.subtract)
    return scale, bias


def build():
    nc = bacc.Bacc("TRN2", target_bir_lowering=False, debug=False,
                   num_devices=N_CORES)
    x_ap = nc.dram_tensor("x", [N_PER, C, H, W], F32, kind="ExternalInput").ap()
    w1_ap = nc.dram_tensor("w1", [C, C, 3, 3], F32, kind="ExternalInput").ap()
    w2_ap = nc.dram_tensor("w2", [C, C, 3, 3], F32, kind="ExternalInput").ap()
    g1_ap = nc.dram_tensor("gamma1", [C], F32, kind="ExternalInput").ap()
    b1_ap = nc.dram_tensor("beta1", [C], F32, kind="ExternalInput").ap()
    g2_ap = nc.dram_tensor("gamma2", [C], F32, kind="ExternalInput").ap()
    b2_ap = nc.dram_tensor("beta2", [C], F32, kind="ExternalInput").ap()
    out_ap = nc.dram_tensor("out", [N_PER, C, H, W], F32,
                            kind="ExternalOutput").ap()
    x_r = x_ap.rearrange("n c h w -> n c h w")
    out_f = out_ap.rearrange("n c h w -> n c (h w)")
    RG = [list(range(N_CORES))]

    with tile.TileContext(nc) as tc:
        with tc.tile_pool(name="consts", bufs=1) as consts, \
             tc.tile_pool(name="T", bufs=N_PER) as pool_T, \
             tc.tile_pool(name="wq", bufs=1) as wqp, \
             tc.tile_pool(name="pads", bufs=N_PER) as padhl, \
             tc.tile_pool(name="apad", bufs=2) as apadp, \
             tc.tile_pool(name="xio", bufs=3) as xio, \
             tc.tile_pool(name="y1", bufs=3) as y1p, \
             tc.tile_pool(name="u", bufs=3) as up, \
             tc.tile_pool(name="xb", bufs=3) as xbp, \
             tc.tile_pool(name="of", bufs=3) as outp, \
             tc.tile_pool(name="psum", bufs=3, space="PSUM") as psum, \
             tc.tile_pool(name="ptr", bufs=2, space="PSUM") as ptr, \
             tc.tile_pool(name="stats", bufs=1) as statsp, \
             tc.tile_pool(name="dram", bufs=1, space="DRAM") as dram:

            # weight DMAs issue first, w1 split across the sync and scalar
            # queues so its halves land in parallel (w1 gates conv1's start)
            K9 = C * 9
            wsb = wqp.tile([C, 2 * K9], F32, tag="wsb")
            w1f = w1_ap.rearrange("o i kh kw -> o (i kh kw)")
            nc.sync.dma_start(wsb[:, 0:K9 // 2], w1f[:, 0:K9 // 2])
            nc.scalar.dma_start(wsb[:, K9 // 2:K9], w1f[:, K9 // 2:])
            nc.scalar.dma_start(wsb[:, K9:],
                                w2_ap.rearrange("o i kh kw -> o (i kh kw)"))

            ident = consts.tile([C, C], BF16, tag="ident")
            make_identity(nc, ident[:])
            lhsT1 = consts.tile([C, 9, C], F16, tag="lhsT1")
            # conv2 weights: 4 DoubleRow pairs + 1 single, fp8
            lhsT2p = consts.tile([C, 4, 2, C], FP8, tag="lhsT2p")
            lhsT2s = consts.tile([C, C], FP8, tag="lhsT2s")

            # preload the sqrt_and_others ACT table (covers Sqrt, Relu and
            # Copy - every activation this kernel uses) so the post-AllGather
            # Sqrt never stalls on a mid-kernel table load
            rsq0 = consts.tile([C, 1], F32, tag="rsq0")
            nc.vector.memset(rsq0[:], 1.0)
            rsq1 = consts.tile([C, 1], F32, tag="rsq1")
            nc.scalar.activation(rsq1[:], rsq0[:], AF.Sqrt, bias=0.0,
                                 scale=1.0)

            # non-Copy activation biases must be APs: small memset consts
            nm16 = consts.tile([C, 1], F32, tag="nm16")
            nc.vector.memset(nm16[:], -MAGIC16)
            p3b = consts.tile([C, 1], F32, tag="p3b")
            nc.vector.memset(p3b[:], P3BIAS)
            epst1 = consts.tile([C, 1], F32, tag="epst1")
            nc.vector.memset(epst1[:], EPS1)
            epst2 = consts.tile([C, 1], F32, tag="epst2")
            nc.vector.memset(epst2[:], EPS2)

            # lhsT copies ride the DVE queue (the ACT queue carries the
            # image-prep copies, which wait on image DMAs)
            def copy1(t, pst):
                nc.vector.tensor_copy(lhsT1[:, t, :], pst[:])

            def copy2(t, pst):
                if t < 8:
                    nc.vector.tensor_copy(lhsT2p[:, t // 2, t % 2, :], pst[:])
                else:
                    nc.vector.tensor_copy(lhsT2s[:], pst[:])

            def prep_image(i):
                """Pad tile holds 15*x in f16: the x15 scale folds into the
                bn1 affine (EPS1 is pre-scaled by 225^2) and the tile doubles
                as the phase-3 residual source (15*x is what the residual
                needs on the code scale)."""
                xp = padhl.tile([C, PW, PW], F16, tag="pad")
                _border_zero(nc, xp)
                for h in range(2):
                    xs = xio.tile([C, 28, W], F32, tag="xio")
                    q = nc.sync if h == 0 else nc.scalar
                    q.dma_start(xs[:], x_r[i, :, 28 * h:28 * (h + 1), :])
                    nc.scalar.activation(
                        xp[:, 1 + 28 * h:1 + 28 * (h + 1), 1:57], xs[:],
                        AF.Copy, bias=0.0, scale=15.0)
                return xp

            g15_1, b15_1 = _load_gb(nc, consts, g1_ap, b1_ap, "1")
            g15_2, b15_2 = _load_gb(nc, consts, g2_ap, b2_ap, "2")

            stats1 = statsp.tile([C, NREC, 6], F32, tag="stats1")
            T1 = []

            def conv_pairs(Ti, mm_fn, stats, i):
                """Run the 7 chunks of one image as 3 PSUM pairs + 1 single;
                evacuate each PSUM tile with one ACT copy (f32 -> f16) and
                record bn_stats per chunk.  Stats normally read the f16
                image (cheap SBUF access); the LAST image reads the f32 PSUM
                instead so the AllGather input never waits on the ACT evac
                (the f16 rounding is mean-preserving to ~2^-11 - far below
                the stats tolerance)."""
                Tir = Ti.rearrange("c (p q) -> c p q", q=CHN)
                for pk in range(4):
                    ps = psum.tile([C, 1024], F32, tag="mm")
                    nch = 2 if pk < 3 else 1
                    for j in range(nch):
                        mm_fn(ps[:, j * 512:j * 512 + CHN], 2 * pk + j)
                    pv = ps.rearrange("c (two x) -> c two x", x=512)
                    if i == N_PER - 1:
                        for j in range(nch):
                            nc.vector.bn_stats(
                                stats[:, i * NCHUNK + 2 * pk + j, :],
                                ps[:, j * 512:j * 512 + CHN])
                    nc.scalar.copy(Tir[:, 2 * pk:2 * pk + nch, :],
                                   pv[:, 0:nch, 0:CHN])
                    if i < N_PER - 1:
                        for j in range(nch):
                            nc.vector.bn_stats(
                                stats[:, i * NCHUNK + 2 * pk + j, :],
                                Tir[:, 2 * pk + j, :])
                return Tir

            def conv1_image(i, xp):
                Ti = pool_T.tile([C, HW], F16, tag="T")
                T1.append(Ti)

                def mm1(out_ap2, ck):
                    r0 = ck * RCH
                    for k, (ky, kx) in enumerate(TAPS):
                        rhs = xp[:, r0 + ky:r0 + ky + RCH, kx:kx + W]
                        nc.tensor.matmul(out_ap2, lhsT1[:, k, :], rhs,
                                         start=(k == 0), stop=(k == 8))

                conv_pairs(Ti, mm1, stats1, i)
                if i == 6:
                    # mid-flight cross-core sync to absorb skew before AG1
                    ccs = dram.tile([C, 6], F32, tag="ccs")
                    ccso = dram.tile([C, 6], F32, tag="ccso")
                    nc.sync.dma_start(ccs[:], stats1[:, 6 * NCHUNK, :])
                    nc.gpsimd.collective_compute(
                        "AllReduce", ALU.add, replica_groups=RG,
                        ins=[ccs.opt()], outs=[ccso.opt()])

            # prep image 0 first (only DMA/ACT/DVE), then quantize w1 so
            # conv1 can start as soon as lhsT1 is ready.  The warm-up
            # collective trigger goes AFTER wquant's gpsimd reduces (the
            # trigger blocks the gpsimd queue until the NEFF init barrier).
            xp0 = prep_image(0)
            gmax1 = _quant_stats(nc, consts, wsb[:, 0:C * 9], "1")
            gmax2 = _quant_stats(nc, consts, wsb[:, C * 9:], "2")

            ccwi = dram.tile([C, 1], F32, tag="ccwi")
            ccwo = dram.tile([C, 1], F32, tag="ccwo")
            nc.gpsimd.dma_start(ccwi[:], gmax2[:])
            nc.gpsimd.collective_compute(
                "AllReduce", ALU.add, replica_groups=RG,
                ins=[ccwi.opt()], outs=[ccwo.opt()])

            # w1's DVE chain gates conv1's start; w2's chain runs in DVE
            # slack during images 0-1 and its PE transposes enter the
            # in-order tensor queue after image 2's matmuls.
            with nc.named_scope("wquant1"):
                wi1 = _quant_chain(nc, wqp, consts, wsb[:, 0:C * 9],
                                   gmax1, "1")
                _quant_transposes(nc, ptr, ident, wi1, copy1)
            xp_next = prep_image(1)
            with nc.named_scope("conv1_img0"):
                conv1_image(0, xp0)
            with nc.named_scope("wquant2_chain"):
                wi2 = _quant_chain(nc, wqp, consts, wsb[:, C * 9:],
                                   gmax2, "2")

            # ---------------- phase 1: conv1 + stats ----------------
            st1a = None
            pads = [xp0, xp_next]
            with nc.named_scope("conv1"):
                for i in range(1, N_PER):
                    xp = pads[i]
                    if i < N_PER - 1:
                        pads.append(prep_image(i + 1))
                    conv1_image(i, xp)
                    if i == 2:
                        with nc.named_scope("wquant2_tr"):
                            _quant_transposes(nc, ptr, ident, wi2, copy2)
                    if i == 6:
                        # aggregate images 0-6 while image 7 is convolving
                        st1a = _stats_contrib(
                            nc, statsp, stats1[:, 0:7 * NCHUNK, :],
                            float(7 * HW) / N_BATCH, "1a")

            # allreduce stats 1 (image 7's records + the precomputed rest)
            st1b = _stats_contrib(nc, statsp, stats1[:, 7 * NCHUNK:, :],
                                  float(HW) / N_BATCH, "1b")
            st1 = statsp.tile([C, 2], F32, tag="st1")
            nc.vector.tensor_tensor(st1[:], st1a[:], st1b[:], ALU.add)
            rst1 = _ag_sum(nc, statsp, dram, st1, RG, "1")
            sc1, bi1 = _bn_vectors(nc, consts, rst1, g15_1, b15_1, epst1, "1")

            # ---------------- phase 2: act1 + conv2 + stats ----------------
            stats2 = statsp.tile([C, NREC, 6], F32, tag="stats2")
            T2 = []

            def act1_image(i):
                """ap_t = fp8 codes of act1(bn1(T1[i])), in two half-image
                DVE chains: y1 = sc1*T1+bi1 (4x f16), u = min(y1,15)+1024
                (4x f16, the cast to f16 IS the round-to-nearest-even),
                then ACT Relu(u-1024) -> fp8."""
                ap_t = apadp.tile([C, PW, PW], FP8, tag="apad")
                _border_zero(nc, ap_t)
                T1r = T1[i].rearrange("c (g f) -> c g f", g=2)
                for h in range(2):
                    y1 = y1p.tile([C, HHW], F16, tag="y1")
                    nc.vector.tensor_scalar(y1[:], T1r[:, h, :], sc1[:, 0:1],
                                            bi1[:, 0:1], ALU.mult, ALU.add)
                    u = up.tile([C, HHW], F16, tag="u")
                    nc.vector.tensor_scalar(u[:], y1[:], 15.0, MAGIC16,
                                            ALU.min, ALU.add)
                    ur = u.rearrange("c (h w) -> c h w", w=W)
                    nc.scalar.activation(
                        ap_t[:, 1 + 28 * h:1 + 28 * (h + 1), 1:57], ur,
                        AF.Relu, bias=nm16[:, 0:1], scale=1.0)
                return ap_t

            def conv2_image(i, ap_t):
                Ti2 = pool_T.tile([C, HW], F16, tag="T")
                T2.append(Ti2)

                def mm2(out_ap2, ck):
                    r0 = ck * RCH
                    for p in range(4):
                        nc.tensor.matmul(out_ap2, lhsT2p[:, p, :, :],
                                         _pair_rhs(ap_t, r0, p),
                                         start=(p == 0), stop=False,
                                         perf_mode=DR)
                    rhs8 = ap_t[:, r0 + 2:r0 + 2 + RCH, 2:2 + W]
                    nc.tensor.matmul(out_ap2, lhsT2s[:], rhs8,
                                     start=False, stop=True)

                conv_pairs(Ti2, mm2, stats2, i)

            st2a = None
            with nc.named_scope("act1_conv2"):
                apts = [act1_image(0)]
                for i in range(N_PER):
                    if i < N_PER - 1:
                        apts.append(act1_image(i + 1))
                    conv2_image(i, apts[i])
                    if i == 6:
                        # mid-flight cross-core sync to absorb skew before AG2
                        ccs2 = dram.tile([C, 6], F32, tag="ccs2")
                        ccso2 = dram.tile([C, 6], F32, tag="ccso2")
                        nc.sync.dma_start(ccs2[:], stats2[:, 6 * NCHUNK, :])
                        nc.gpsimd.collective_compute(
                            "AllReduce", ALU.add, replica_groups=RG,
                            ins=[ccs2.opt()], outs=[ccso2.opt()])
                        st2a = _stats_contrib(
                            nc, statsp, stats2[:, 0:7 * NCHUNK, :],
                            float(7 * HW) / N_BATCH, "2a")

            # allreduce stats 2 (image 7's records + the precomputed rest)
            st2b = _stats_contrib(nc, statsp, stats2[:, 7 * NCHUNK:, :],
                                  float(HW) / N_BATCH, "2b")
            st2 = statsp.tile([C, 2], F32, tag="st2")
            nc.vector.tensor_tensor(st2[:], st2a[:], st2b[:], ALU.add)
            rst2 = _ag_sum(nc, statsp, dram, st2, RG, "2")
            sc2, bi2 = _bn_vectors(nc, consts, rst2, g15_2, b15_2, epst2, "2")

            # ------------- phase 3: bn2 + residual + act -> out -------------
            # xb = sc2*T2 + bi2 (4x f16 dual-AP tensor_scalar);
            # y15 = xb + pad (pad holds 15*x; 2x f16 TT, some halves on the
            # otherwise-idle gpsimd);
            # u = min(y15,15) + 1024 -> f16 (the cast rounds);
            # out = Relu(u/15 - 1024/15) -> f32, stored on 3 DMA queues.
            oqs = [nc.sync, nc.gpsimd, nc.scalar]
            with nc.named_scope("final"):
                for i in range(N_PER):
                    T2r = T2[i].rearrange("c (g f) -> c g f", g=2)
                    padi = pads[i]
                    for h in range(2):
                        xb = xbp.tile([C, HHW], F16, tag="xb")
                        nc.vector.tensor_scalar(
                            xb[:], T2r[:, h, :], sc2[:, 0:1], bi2[:, 0:1],
                            ALU.mult, ALU.add)
                        xbr = xb.rearrange("c (h w) -> c h w", w=W)
                        padsl = padi[:, 1 + 28 * h:1 + 28 * (h + 1), 1:57]
                        tteng = nc.gpsimd if (2 * i + h) % 4 == 3 else \
                            nc.vector
                        tteng.tensor_tensor(xbr, xbr, padsl, ALU.add)
                        u = up.tile([C, HHW], F16, tag="u")
                        nc.vector.tensor_scalar(u[:], xb[:], 15.0, MAGIC16,
                                                ALU.min, ALU.add)
                        of = outp.tile([C, HHW], F32, tag="of")
                        nc.scalar.activation(of[:], u[:], AF.Relu,
                                             bias=p3b[:, 0:1], scale=INV15)
                        oqs[(2 * i + h) % 3].dma_start(
                            out_f[i, :, HHW * h:HHW * (h + 1)], of[:])

    nc.compile()
    return nc


def kernel(x, w1, w2, gamma1, beta1, gamma2, beta2):
    if "nc" not in _CACHED:
        _CACHED["nc"] = build()
    nc = _CACHED["nc"]
    x = np.ascontiguousarray(x, dtype=np.float32)
    shard = x.reshape(N_CORES, N_PER, C, H, W)
    common = {
        "w1": np.ascontiguousarray(w1, np.float32),
        "w2": np.ascontiguousarray(w2, np.float32),
        "gamma1": np.ascontiguousarray(gamma1, np.float32),
        "beta1": np.ascontiguousarray(beta1, np.float32),
        "gamma2": np.ascontiguousarray(gamma2, np.float32),
        "beta2": np.ascontiguousarray(beta2, np.float32),
    }
    in_maps = [{"x": shard[i], **common} for i in range(N_CORES)]
    old_m = nc.m
    nc.m = get_hw_module(nc.m)
    try:
        res = run_bass_kernel_spmd(nc, in_maps, core_ids=list(range(N_CORES)))
    finally:
        nc.m = old_m
    out = np.concatenate([res.results[i]["out"] for i in range(N_CORES)], axis=0)
    return out.astype(np.float32)
